# revision 28
# baseline (speedup 1.0000x reference)
"""Fused LayerNorm + causal multi-head attention for Trainium2, 8 NeuronCores.

Problem: x[2,2048,1024] -> LN -> qkv proj (w_qkv[1024,3072]) -> 16-head causal
attention (d=64) -> out proj (w_out[1024,1024]).

Sharding (no cross-core communication):
  core c = b*4 + hg   (b in {0,1} batches, hg in {0..3} head-groups of 4 heads)
  Each core computes its batch's LN + its 4 heads' qkv/attention + a partial
  out-projection (its 256 rows of w_out). Host sums the 4 partials per batch.

Device algorithm (transposed layout: features on partitions, sequence on the
free axis; everything bf16 on the PE so matmuls pipeline at stream rate):
  A. stats: per 512-col block, colsums of x / x^2 via ones-matmuls (t-outer so
     sigma chains overlap later stats; x^2 on ScalarE); LN folded into the QKV
     matmul via a merged K=2 bf16 aug matmul (rows [-mean; std] x [u; vb]);
     the rs[n] factor multiplies the psum in the epilogue (a_bc, bf16).
     ct order v,v,q,k,q,k so V is ready before attention starts.
  B. v -> natural layout via DMA xbar transpose (no PE/DVE transpose work)
  C. attention, head PAIRS via PE row tiling, i-block outer: per (ib, pair),
     j-tiles stream K=64 QK matmuls for both heads concurrently into the two
     halves of a [128,1024] 2-bank psum; ONE wide exp per j-tile covers both
     heads; causal diag masked in-place on GpSimd; PV accumulates [66,512]
     per head (64 v-rows + 2 ones rows = softmax denominator); normalize =
     bf16 den broadcast via K=1 matmul + fast reciprocal + multiply.
     Normalizes and the per-i-block out-projection are DEFERRED and emitted
     one-per-j-tile inside the (ScalarE-paced) attention stream so the PE
     always has filler work and the HAM clock stays warm.
"""
import os
import sys

for _p in ("/opt/trn_rl_repo", "/root/.axon_site/_ro/trn_rl_repo"):
    if os.path.isdir(_p) and _p not in sys.path:
        sys.path.insert(0, _p)

import numpy as np

import concourse.bass as bass  # noqa: F401
import concourse.mybir as mybir
import concourse.tile as tile
from concourse import bacc
from concourse.bass_utils import run_bass_kernel_spmd

F32 = mybir.dt.float32
BF16 = mybir.dt.bfloat16
MUL = mybir.AluOpType.mult
ADD = mybir.AluOpType.add
SUB = mybir.AluOpType.subtract
AF = mybir.ActivationFunctionType

B, N, DIM = 2, 2048, 1024
HEADS, DH = 16, 64
HPC = 4            # heads per core
CD = HPC * DH      # 256 output channels per core
SCALE = DH ** -0.5
EPS = 1e-5
NT = N // 512      # 4 col-blocks of 512
NK = DIM // 128    # 8 contraction chunks
NROW = N // 128    # 16 row tiles of 128
VW = DH + 2        # 66: v columns + 2 ones columns per head

CT_ORDER = [4, 5, 0, 2, 1, 3]   # v01 v23 q01 k01 q23 k23


def _build():
    nc = bacc.Bacc("TRN2", target_bir_lowering=False, debug=False)

    xT_ext = nc.declare_dram_parameter("xT", [DIM, N], BF16, isOutput=False)
    w_ext = nc.declare_dram_parameter("wqkv", [DIM, 3 * CD], BF16, isOutput=False)
    uv_ext = nc.declare_dram_parameter("uv", [2, 3 * CD], BF16, isOutput=False)
    wo_ext = nc.declare_dram_parameter("wout", [CD, DIM], BF16, isOutput=False)
    ones_ext = nc.declare_dram_parameter("ones", [128, 128], BF16, isOutput=False)
    mask_ext = nc.declare_dram_parameter("mask", [128, 256], BF16, isOutput=False)
    bm_ext = nc.declare_dram_parameter("bm", [2, 128], BF16, isOutput=False)
    out_ext = nc.declare_dram_parameter("out", [N, DIM], BF16, isOutput=True)

    with tile.TileContext(nc) as tc:
        with (
            nc.allow_low_precision(reason="bf16 everywhere; psum stays f32"),
            tc.tile_pool(name="persist", bufs=1) as pp,
        ):
            ones_b = pp.tile([128, 128], BF16, tag="ones_b")
            mask_t = pp.tile([128, 256], BF16, tag="mask")
            bm_t = pp.tile([2, 128], BF16, tag="bm")
            nc.sync.dma_start(bm_t[:], bm_ext[:])
            wo_t = pp.tile([128, 2, DIM], BF16, tag="wo")
            nc.sync.dma_start(ones_b[:], ones_ext[:])
            nc.sync.dma_start(mask_t[:], mask_ext[:])
            wo_d = wo_ext[:].rearrange("(c p) m -> p c m", p=128)
            nc.sync.dma_start(wo_t[:, 0, :], wo_d[:, 0, :])
            nc.sync.dma_start(wo_t[:, 1, :], wo_d[:, 1, :])

            # qkvT tiles: [q01 q23 k01 k23 v01 v23], each [128, N]
            qkvT = [pp.tile([128, N], BF16, tag=f"qkvT{i}", name=f"qkvT{i}")
                    for i in range(6)]
            a_bc = pp.tile([128, N], BF16, tag="a_bc")      # rs[n] broadcast
            # merged aug rhs: partition0 = b2 = -mean, partition1 = std
            rows_aug = pp.tile([2, N], BF16, tag="rows_aug")
            v_nat = pp.tile([128, NROW, HPC * VW], BF16, tag="v_nat")
            oT = [pp.tile([128, N], BF16, tag=f"oT{i}", name=f"oT{i}")
                  for i in range(2)]

            # ---------------- phase A: stats + qkv projection ----------------
            with (
                tc.tile_pool(name="pA", bufs=1) as pa,
                tc.tile_pool(name="pAs", bufs=1) as pas,
                tc.tile_pool(name="pB", bufs=2) as pb,
                tc.tile_pool(name="psA", bufs=1, space="PSUM") as psa,
            ):
                xT = pa.tile([128, NK, N], BF16, tag="xT")
                xsq = pa.tile([128, NK, N], BF16, tag="xsq")
                wq = pa.tile([128, NK, 3 * CD], BF16, tag="wq")
                uv_t = pa.tile([2, 3 * CD], BF16, tag="uv")
                nc.sync.dma_start(uv_t[:], uv_ext[:])
                xT_d = xT_ext[:].rearrange("(c p) n -> p c n", p=128)
                w_d = w_ext[:].rearrange("(c p) m -> p c m", p=128)
                for k in range(NK):
                    nc.sync.dma_start(xT[:, k, :], xT_d[:, k, :])
                    nc.sync.dma_start(wq[:, k, :], w_d[:, k, :])
                    nc.vector.tensor_tensor(xsq[:, k, :], xT[:, k, :],
                                            xT[:, k, :], op=MUL)

                scr = pas.tile([2, 3, N], F32, tag="scr")  # mean/vpre/spare
                rs_f = pas.tile([1, N], F32, tag="rs_f")
                rs_b = pas.tile([1, N], BF16, tag="rs_b")

                # stats + sigma chain, t-outer
                for t in range(NT):
                    cs = slice(t * 512, (t + 1) * 512)
                    ps_s = psa.tile([128, 512], F32, tag="st_s", bufs=1)
                    ps_q = psa.tile([128, 512], F32, tag="st_q", bufs=1)
                    for k in range(NK):
                        nc.tensor.matmul(ps_s[:], ones_b[:], xT[:, k, cs],
                                         start=(k == 0), stop=(k == NK - 1),
                                         skip_group_check=True)
                        nc.tensor.matmul(ps_q[:], ones_b[:], xsq[:, k, cs],
                                         start=(k == 0), stop=(k == NK - 1),
                                         skip_group_check=True)
                    # sigma chain on partitions 0:2 (psum rows replicated)
                    m2 = scr[0:2, 0, cs]
                    vp = scr[0:2, 1, cs]
                    mm2 = scr[0:2, 2, cs]
                    nc.scalar.activation(m2, ps_s[0:2, :], AF.Copy,
                                         scale=1.0 / DIM)
                    nc.scalar.activation(vp, ps_q[0:2, :], AF.Copy,
                                         scale=1.0 / DIM, bias=EPS)
                    nc.vector.tensor_tensor(mm2, m2, m2, op=MUL)
                    nc.vector.tensor_tensor(vp, vp, mm2, op=SUB)  # var
                    # aug rows: std = sqrt(var) into rows 0:2, then overwrite
                    # row 0 with b2 = -mean (ops must start at partition 0)
                    nc.scalar.activation(rows_aug[0:2, cs], scr[0:2, 1, cs],
                                         AF.Sqrt)
                    nc.scalar.activation(rows_aug[0:1, cs], ps_s[0:1, :],
                                         AF.Copy, scale=-1.0 / DIM)
                    # rs = 1/std @p0 -> bf16 (for the a_bc broadcast)
                    nc.scalar.activation(rs_f[0:1, cs], scr[0:1, 1, cs],
                                         AF.Sqrt)
                    nc.vector.reciprocal_approx_fast(scr[0:1, 0, cs],
                                                     rs_f[0:1, cs])
                    nc.vector.tensor_copy(rs_b[0:1, cs], scr[0:1, 0, cs])

                def _emit_ab(t):
                    cs = slice(t * 512, (t + 1) * 512)
                    ab_ps = psa.tile([128, 512], F32, tag="qkv", bufs=6)
                    nc.tensor.matmul(ab_ps[:], ones_b[0:1, :], rs_b[0:1, cs],
                                     start=True, stop=True,
                                     skip_group_check=True)
                    nc.vector.tensor_copy(a_bc[:, cs], ab_ps[:])

                def _emit_ct(ct):
                    ms = slice(ct * 128, (ct + 1) * 128)
                    ps = [psa.tile([128, 512], F32, tag="qkv", bufs=6,
                                   name=f"qkv_{ct}_{t}")
                          for t in range(NT)]
                    for k in range(NK):
                        for t in range(NT):
                            cs = slice(t * 512, (t + 1) * 512)
                            nc.tensor.matmul(ps[t][:], wq[:, k, ms],
                                             xT[:, k, cs], start=(k == 0),
                                             stop=False, skip_group_check=True)
                    return ps, ms

                def _emit_ct_tail(ct, ps, ms, after_t=None):
                    for t in range(NT):
                        cs = slice(t * 512, (t + 1) * 512)
                        nc.tensor.matmul(ps[t][:], uv_t[0:2, ms],
                                         rows_aug[0:2, cs], start=False,
                                         stop=True, skip_group_check=True)
                        nc.vector.tensor_tensor(qkvT[ct][:, cs], ps[t][:],
                                                a_bc[:, cs], op=MUL)
                        if after_t is not None:
                            after_t(t)

                def _emit_vnat(hp):
                    # heads 2hp, 2hp+1 from qkvT[4+hp] via DMA xbar transpose
                    for off in (0, 64):
                        h = 2 * hp + off // 64
                        vs = pb.tile([128, NROW, DH], BF16, tag="vscr",
                                     name=f"vscr{h}")
                        nc.sync.dma_start_transpose(
                            vs[:], qkvT[4 + hp][off:off + 64, :])
                        nc.vector.tensor_copy(
                            v_nat[:, :, h * VW:h * VW + DH], vs[:])
                        nc.vector.memset(
                            v_nat[:, :, h * VW + DH:h * VW + VW], 1.0)

                # ct4 k-chains right after stats (no sigma dependency,
                # covers the serial sigma tails); a_bc broadcasts + tails
                # trickle in as sigma[t] complete
                ps4, ms4 = _emit_ct(4)
                _emit_ab(0)
                _emit_ab(1)
                # epi-t needs ab[t]; ab2/ab3 slot in as epi-t frees banks
                _emit_ct_tail(4, ps4, ms4,
                              after_t=lambda t: _emit_ab(t + 2)
                              if t < 2 else None)
                for ct in CT_ORDER[1:]:
                    ps_c, ms_c = _emit_ct(ct)
                    _emit_ct_tail(ct, ps_c, ms_c)
                    if ct == 5:
                        _emit_vnat(0)
                        _emit_vnat(1)

            # ------- phase C: attention + interleaved out-projection ---------
            with (
                tc.tile_pool(name="pC", bufs=1) as pc,
                tc.tile_pool(name="psC", bufs=1, space="PSUM") as psc,
            ):
                pending_norm = []  # deferred normalizes (drain at jt0)
                pending_op = []    # deferred out-proj tiles (one per j-tile)

                def _norm(P, ib, o_ps, off):
                    """Normalize one head: den bcast, recip, mul."""
                    isl = slice(ib * 512, (ib + 1) * 512)
                    dnb = pc.tile([128, 512], BF16, tag="dnb", bufs=2)
                    nc.vector.tensor_copy(dnb[64:65, :], o_ps[64:65, :])
                    rb_t = psc.tile([128, 1024], F32, tag="s", bufs=3,
                                    name=f"rb{P}{ib}{off}")
                    rb_ps = rb_t[0:64, 0:512]
                    nc.tensor.matmul(rb_ps, ones_b[64:65, 0:64],
                                     dnb[64:65, :], start=True, stop=True,
                                     skip_group_check=True)
                    rdb = pc.tile([64, 512], F32, tag="rdb", bufs=2)
                    nc.vector.reciprocal_approx_fast(rdb[:], rb_ps)
                    if off == 0:
                        nc.vector.tensor_tensor(oT[P][0:64, isl],
                                                o_ps[0:64, :], rdb[:], op=MUL)
                    else:
                        osh = pc.tile([64, 512], BF16, tag="osh", bufs=2)
                        nc.vector.tensor_tensor(osh[:], o_ps[0:64, :],
                                                rdb[:], op=MUL)
                        nc.sync.dma_start(oT[P][64:128, isl], osh[:])

                def _outproj(t, use_act=False):
                    """One 128-token row tile of the partial out-projection.
                    use_act routes the psum->bf16 copy to ScalarE (idle at
                    the kernel tail while DVE finishes the last norms)."""
                    rsl = slice(t * 128, (t + 1) * 128)
                    op = psc.tile([128, 1024], F32, tag="s", bufs=3,
                                  name=f"op{t}")
                    for c in range(2):
                        for mt in range(2):
                            nc.tensor.matmul(
                                op[:, mt * 512:(mt + 1) * 512],
                                oT[c][:, rsl], wo_t[:, c, mt * 512:(mt + 1) * 512],
                                start=(c == 0), stop=(c == 1),
                                skip_group_check=True)
                    ost = pc.tile([128, 1024], BF16, tag="ost", bufs=2,
                                  name=f"ost{t}")
                    if use_act:
                        nc.scalar.activation(ost[:], op[:], AF.Copy)
                    else:
                        nc.vector.tensor_copy(ost[:], op[:])
                    nc.sync.dma_start(out_ext[rsl, :], ost[:])

                for ib in (3, 2, 1, 0):
                    i0 = ib * 512
                    n_jt = 4 * (ib + 1)
                    for P in range(2):
                        qT = qkvT[P]
                        kT = qkvT[2 + P]
                        hA, hB = 2 * P, 2 * P + 1
                        o_A = psc.tile([VW, 512], F32, tag="o", bufs=2,
                                       name=f"oA{ib}{P}")
                        o_B = psc.tile([VW, 512], F32, tag="o", bufs=2,
                                       name=f"oB{ib}{P}")
                        pend_pv = None

                        for jt in range(n_jt):
                            j0 = jt * 128
                            so = max(0, j0 - i0)
                            s2 = psc.tile([128, 1024], F32, tag="s", bufs=3,
                                          name=f"s{ib}{P}{jt}")
                            # QK pair: A rows 0:64 @ pos(0,0), B rows 64:128
                            nc.tensor.matmul(
                                s2[:, so:512], kT[0:64, j0:j0 + 128],
                                qT[0:64, i0 + so:i0 + 512],
                                start=True, stop=True, skip_group_check=True)
                            nc.tensor.matmul(
                                s2[:, 512 + so:1024], kT[64:128, j0:j0 + 128],
                                qT[64:128, i0 + so:i0 + 512],
                                start=True, stop=True, skip_group_check=True)
                            e2 = pc.tile([128, 1024], BF16, tag="e", bufs=3,
                                         name=f"e{ib}{P}{jt}")
                            nc.scalar.activation(e2[:, so:1024],
                                                 s2[:, so:1024], AF.Exp)
                            if j0 >= i0:  # diagonal tile: mask both heads
                                ev = e2[:].rearrange(
                                    "p (a b) -> p a b", b=512)[:, :, so:so + 128]
                                mv = mask_t[:].rearrange(
                                    "p (a b) -> p a b", b=128)
                                nc.vector.tensor_tensor(ev, ev, mv, op=MUL)

                            def _pv(so_, jt_, e2_, first, last):
                                nc.tensor.matmul(
                                    o_A[:, so_:512],
                                    v_nat[:, jt_, hA * VW:hA * VW + VW],
                                    e2_[:, so_:512], start=first, stop=last,
                                    skip_group_check=True)
                                nc.tensor.matmul(
                                    o_B[:, so_:512],
                                    v_nat[:, jt_, hB * VW:hB * VW + VW],
                                    e2_[:, 512 + so_:1024], start=first,
                                    stop=last, skip_group_check=True)

                            if pend_pv is not None:
                                pend_pv()
                            pend_pv = (lambda a=so, b=jt, c=e2,
                                       f=(jt == 0), l=(jt == n_jt - 1):
                                       _pv(a, b, c, f, l))
                            if jt == 0:
                                # norms of the previous block: after this
                                # block's first QK/exp (keeps ScalarE fed),
                                # before its first PV reuses the o-ring slots
                                while pending_norm:
                                    pending_norm.pop(0)()
                            # deferred PE filler, spread thinly so the PE
                            # never starves ScalarE of QK psums
                            elif pending_op and (jt % 4 == 2
                                                 or len(pending_op) > 4):
                                pending_op.pop(0)()
                        pend_pv()
                        pending_norm.append(
                            lambda P_=P, ib_=ib, o_=o_A: _norm(P_, ib_, o_, 0))
                        pending_norm.append(
                            lambda P_=P, ib_=ib, o_=o_B: _norm(P_, ib_, o_, 64))
                    # out-projection row tiles for this i-block (deferred)
                    for t in range(4 * ib, 4 * ib + 4):
                        pending_op.append(
                            lambda t_=t, a=False: _outproj(t_, a))
                while pending_norm:
                    pending_norm.pop(0)()
                while pending_op:
                    t_fn = pending_op.pop(0)
                    t_fn.__defaults__ = (t_fn.__defaults__[0], True)
                    t_fn()

    nc.compile()
    return nc


_NC_CACHE = {}


def _get_nc():
    if "nc" not in _NC_CACHE:
        _NC_CACHE["nc"] = _build()
    return _NC_CACHE["nc"]


def _prep_in_maps(x, ln_w, ln_b, w_qkv, w_out):
    import ml_dtypes
    _bf = ml_dtypes.bfloat16
    x = np.asarray(x, dtype=np.float32)
    ln_w = np.asarray(ln_w, dtype=np.float32)
    ln_b = np.asarray(ln_b, dtype=np.float32)
    w_qkv = np.asarray(w_qkv, dtype=np.float32)
    w_out = np.asarray(w_out, dtype=np.float32)

    ones = np.ones((128, 128), dtype=_bf)
    # mask[jp, ii] = 1 iff jp <= ii (keep j <= i), doubled side by side so a
    # single strided DVE op masks both heads' diagonal tiles
    mask1 = np.triu(np.ones((128, 128), dtype=np.float32))
    mask = np.concatenate([mask1, mask1], axis=1).astype(_bf)
    # block mask: row0 selects head-A denominator for out partitions 0:64,
    # row1 selects head-B for 64:128
    bm = np.zeros((2, 128), dtype=np.float32)
    bm[0, 0:64] = 1.0
    bm[1, 64:128] = 1.0
    bm = bm.astype(_bf)

    xTs = [np.ascontiguousarray(x[b].T).astype(_bf) for b in range(B)]

    in_maps = []
    for core in range(8):
        b, hg = core // 4, core % 4
        csl = slice(hg * CD, (hg + 1) * CD)
        # raw slices with SCALE folded into q
        w0 = np.concatenate([w_qkv[:, csl] * SCALE,
                             w_qkv[:, DIM + hg * CD:DIM + (hg + 1) * CD],
                             w_qkv[:, 2 * DIM + hg * CD:2 * DIM + (hg + 1) * CD]],
                            axis=1)
        wf = ln_w[:, None] * w0                      # ln_w folded
        u = wf.sum(axis=0)                           # pairs with -mean
        vb = ln_b @ w0                               # pairs with std (ln bias)
        uv = np.stack([u, vb]).astype(_bf)
        in_maps.append({
            "xT": xTs[b],
            "wqkv": wf.astype(_bf),
            "uv": uv,
            "wout": np.ascontiguousarray(w_out[csl, :]).astype(_bf),
            "ones": ones,
            "mask": mask,
            "bm": bm,
        })
    return in_maps


def _combine(results):
    out = np.empty((B, N, DIM), dtype=np.float32)
    for b in range(B):
        acc = results[b * 4]["out"].astype(np.float32)
        for hg in range(1, 4):
            acc = acc + results[b * 4 + hg]["out"].astype(np.float32)
        out[b] = acc
    return out


def kernel(x, ln_w, ln_b, w_qkv, w_out):
    nc = _get_nc()
    in_maps = _prep_in_maps(x, ln_w, ln_b, w_qkv, w_out)
    res = run_bass_kernel_spmd(nc, in_maps, core_ids=list(range(8)))
    return _combine(res.results)


def run_traced(x, ln_w, ln_b, w_qkv, w_out, **kwargs):
    """Run with NTFF profiling; returns (output, BassKernelResults)."""
    nc = _get_nc()
    in_maps = _prep_in_maps(x, ln_w, ln_b, w_qkv, w_out)
    res = run_bass_kernel_spmd(nc, in_maps, core_ids=list(range(8)),
                               trace=True, **kwargs)
    return _combine(res.results), res


# revision 29
# speedup vs baseline: 1.2116x; 1.2116x over previous
"""Fused LayerNorm + causal multi-head attention for Trainium2, 8 NeuronCores.

Problem: x[2,2048,1024] -> LN -> qkv proj (w_qkv[1024,3072]) -> 16-head causal
attention (d=64) -> out proj (w_out[1024,1024]).

Sharding (no cross-core communication):
  core c = b*4 + hg   (b in {0,1} batches, hg in {0..3} head-groups of 4 heads)
  Each core computes its batch's LN + its 4 heads' qkv/attention + a partial
  out-projection (its 256 rows of w_out). Host sums the 4 partials per batch.

Device algorithm (transposed layout: features on partitions, sequence on the
free axis; everything bf16 on the PE so matmuls pipeline at stream rate):
  A. stats: per 512-col block, colsums of x / x^2 via ones-matmuls (t-outer so
     sigma chains overlap later stats; x^2 on ScalarE); LN folded into the QKV
     matmul via a merged K=2 bf16 aug matmul (rows [-mean; std] x [u; vb]);
     the rs[n] factor multiplies the psum in the epilogue (a_bc, bf16).
     ct order v,v,q,k,q,k so V is ready before attention starts.
  B. v -> natural layout via DMA xbar transpose (no PE/DVE transpose work)
  C. attention, head PAIRS via PE row tiling, i-block outer: per (ib, pair),
     j-tiles stream K=64 QK matmuls for both heads concurrently into the two
     halves of a [128,1024] 2-bank psum; ONE wide exp per j-tile covers both
     heads; causal diag masked in-place on GpSimd; PV accumulates [66,512]
     per head (64 v-rows + 2 ones rows = softmax denominator); normalize =
     bf16 den broadcast via K=1 matmul + fast reciprocal + multiply.
     Normalizes and the per-i-block out-projection are DEFERRED and emitted
     one-per-j-tile inside the (ScalarE-paced) attention stream so the PE
     always has filler work and the HAM clock stays warm.
"""
import os
import sys

for _p in ("/opt/trn_rl_repo", "/root/.axon_site/_ro/trn_rl_repo"):
    if os.path.isdir(_p) and _p not in sys.path:
        sys.path.insert(0, _p)

import numpy as np

import concourse.bass as bass  # noqa: F401
import concourse.mybir as mybir
import concourse.tile as tile
from concourse import bacc
from concourse.bass_utils import run_bass_kernel_spmd

F32 = mybir.dt.float32
BF16 = mybir.dt.bfloat16
MUL = mybir.AluOpType.mult
ADD = mybir.AluOpType.add
SUB = mybir.AluOpType.subtract
AF = mybir.ActivationFunctionType

B, N, DIM = 2, 2048, 1024
HEADS, DH = 16, 64
HPC = 4            # heads per core
CD = HPC * DH      # 256 output channels per core
SCALE = DH ** -0.5
EPS = 1e-5
NT = N // 512      # 4 col-blocks of 512
NK = DIM // 128    # 8 contraction chunks
NROW = N // 128    # 16 row tiles of 128
VW = DH + 2        # 66: v columns + 2 ones columns per head

CT_ORDER = [4, 5, 0, 2, 1, 3]   # v01 v23 q01 k01 q23 k23


def _build():
    nc = bacc.Bacc("TRN2", target_bir_lowering=False, debug=False)

    xT_ext = nc.declare_dram_parameter("xT", [DIM, N], BF16, isOutput=False)
    w_ext = nc.declare_dram_parameter("wqkv", [DIM, 3 * CD], BF16, isOutput=False)
    uv_ext = nc.declare_dram_parameter("uv", [2, 3 * CD], BF16, isOutput=False)
    wo_ext = nc.declare_dram_parameter("wout", [CD, DIM], BF16, isOutput=False)
    ones_ext = nc.declare_dram_parameter("ones", [128, 128], BF16, isOutput=False)
    mask_ext = nc.declare_dram_parameter("mask", [128, 256], BF16, isOutput=False)
    bm_ext = nc.declare_dram_parameter("bm", [2, 128], BF16, isOutput=False)
    out_ext = nc.declare_dram_parameter("out", [N, DIM], BF16, isOutput=True)

    with tile.TileContext(nc) as tc:
        with (
            nc.allow_low_precision(reason="bf16 everywhere; psum stays f32"),
            tc.tile_pool(name="persist", bufs=1) as pp,
        ):
            ones_b = pp.tile([128, 128], BF16, tag="ones_b")
            mask_t = pp.tile([128, 256], BF16, tag="mask")
            bm_t = pp.tile([2, 128], BF16, tag="bm")
            nc.sync.dma_start(bm_t[:], bm_ext[:])
            wo_t = pp.tile([128, 2, DIM], BF16, tag="wo")
            nc.sync.dma_start(ones_b[:], ones_ext[:])
            nc.sync.dma_start(mask_t[:], mask_ext[:])
            wo_d = wo_ext[:].rearrange("(c p) m -> p c m", p=128)
            nc.sync.dma_start(wo_t[:, 0, :], wo_d[:, 0, :])
            nc.sync.dma_start(wo_t[:, 1, :], wo_d[:, 1, :])

            # qkvT tiles: [q01 q23 k01 k23 v01 v23], each [128, N]
            qkvT = [pp.tile([128, N], BF16, tag=f"qkvT{i}", name=f"qkvT{i}")
                    for i in range(6)]
            a_bc = pp.tile([128, N], BF16, tag="a_bc")      # rs[n] broadcast
            # merged aug rhs: partition0 = b2 = -mean, partition1 = std
            rows_aug = pp.tile([2, N], BF16, tag="rows_aug")
            v_nat = pp.tile([128, NROW, HPC * VW], BF16, tag="v_nat")
            oT = [pp.tile([128, N], BF16, tag=f"oT{i}", name=f"oT{i}")
                  for i in range(2)]

            # ---------------- phase A: stats + qkv projection ----------------
            with (
                tc.tile_pool(name="pA", bufs=1) as pa,
                tc.tile_pool(name="pAs", bufs=1) as pas,
                tc.tile_pool(name="pB", bufs=2) as pb,
                tc.tile_pool(name="psA", bufs=1, space="PSUM") as psa,
            ):
                xT = pa.tile([128, NK, N], BF16, tag="xT")
                xsq = pa.tile([128, NK, N], BF16, tag="xsq")
                wq = pa.tile([128, NK, 3 * CD], BF16, tag="wq")
                uv_t = pa.tile([2, 3 * CD], BF16, tag="uv")
                nc.sync.dma_start(uv_t[:], uv_ext[:])
                xT_d = xT_ext[:].rearrange("(c p) n -> p c n", p=128)
                w_d = w_ext[:].rearrange("(c p) m -> p c m", p=128)
                for k in range(NK):
                    nc.sync.dma_start(xT[:, k, :], xT_d[:, k, :])
                    nc.sync.dma_start(wq[:, k, :], w_d[:, k, :])
                    nc.vector.tensor_tensor(xsq[:, k, :], xT[:, k, :],
                                            xT[:, k, :], op=MUL)

                scr = pas.tile([2, 3, N], F32, tag="scr")  # mean/vpre/spare
                rs_f = pas.tile([1, N], F32, tag="rs_f")
                rs_b = pas.tile([1, N], BF16, tag="rs_b")

                # stats + sigma chain, t-outer
                for t in range(NT):
                    cs = slice(t * 512, (t + 1) * 512)
                    ps_s = psa.tile([128, 512], F32, tag="st_s", bufs=1)
                    ps_q = psa.tile([128, 512], F32, tag="st_q", bufs=1)
                    for k in range(NK):
                        nc.tensor.matmul(ps_s[:], ones_b[:], xT[:, k, cs],
                                         start=(k == 0), stop=(k == NK - 1),
                                         skip_group_check=True)
                        nc.tensor.matmul(ps_q[:], ones_b[:], xsq[:, k, cs],
                                         start=(k == 0), stop=(k == NK - 1),
                                         skip_group_check=True)
                    # sigma chain on partitions 0:2 (psum rows replicated)
                    m2 = scr[0:2, 0, cs]
                    vp = scr[0:2, 1, cs]
                    mm2 = scr[0:2, 2, cs]
                    nc.scalar.activation(m2, ps_s[0:2, :], AF.Copy,
                                         scale=1.0 / DIM)
                    nc.scalar.activation(vp, ps_q[0:2, :], AF.Copy,
                                         scale=1.0 / DIM, bias=EPS)
                    nc.vector.tensor_tensor(mm2, m2, m2, op=MUL)
                    nc.vector.tensor_tensor(vp, vp, mm2, op=SUB)  # var
                    # aug rows: std = sqrt(var) into rows 0:2, then overwrite
                    # row 0 with b2 = -mean (ops must start at partition 0)
                    nc.scalar.activation(rows_aug[0:2, cs], scr[0:2, 1, cs],
                                         AF.Sqrt)
                    nc.scalar.activation(rows_aug[0:1, cs], ps_s[0:1, :],
                                         AF.Copy, scale=-1.0 / DIM)
                    # rs = 1/std @p0 -> bf16 (for the a_bc broadcast)
                    nc.scalar.activation(rs_f[0:1, cs], scr[0:1, 1, cs],
                                         AF.Sqrt)
                    nc.vector.reciprocal_approx_fast(scr[0:1, 0, cs],
                                                     rs_f[0:1, cs])
                    nc.vector.tensor_copy(rs_b[0:1, cs], scr[0:1, 0, cs])

                def _emit_ab(t):
                    cs = slice(t * 512, (t + 1) * 512)
                    ab_ps = psa.tile([128, 512], F32, tag="qkv", bufs=6)
                    nc.tensor.matmul(ab_ps[:], ones_b[0:1, :], rs_b[0:1, cs],
                                     start=True, stop=True,
                                     skip_group_check=True)
                    nc.vector.tensor_copy(a_bc[:, cs], ab_ps[:])

                def _emit_ct(ct):
                    ms = slice(ct * 128, (ct + 1) * 128)
                    ps = [psa.tile([128, 512], F32, tag="qkv", bufs=6,
                                   name=f"qkv_{ct}_{t}")
                          for t in range(NT)]
                    for k in range(NK):
                        for t in range(NT):
                            cs = slice(t * 512, (t + 1) * 512)
                            nc.tensor.matmul(ps[t][:], wq[:, k, ms],
                                             xT[:, k, cs], start=(k == 0),
                                             stop=False, skip_group_check=True)
                    return ps, ms

                def _emit_ct_tail(ct, ps, ms, after_t=None):
                    for t in range(NT):
                        cs = slice(t * 512, (t + 1) * 512)
                        nc.tensor.matmul(ps[t][:], uv_t[0:2, ms],
                                         rows_aug[0:2, cs], start=False,
                                         stop=True, skip_group_check=True)
                        nc.vector.tensor_tensor(qkvT[ct][:, cs], ps[t][:],
                                                a_bc[:, cs], op=MUL)
                        if after_t is not None:
                            after_t(t)

                def _emit_vnat(hp):
                    # heads 2hp, 2hp+1 from qkvT[4+hp] via DMA xbar transpose
                    for off in (0, 64):
                        h = 2 * hp + off // 64
                        vs = pb.tile([128, NROW, DH], BF16, tag="vscr",
                                     name=f"vscr{h}")
                        nc.sync.dma_start_transpose(
                            vs[:], qkvT[4 + hp][off:off + 64, :])
                        nc.vector.tensor_copy(
                            v_nat[:, :, h * VW:h * VW + DH], vs[:])
                        nc.vector.memset(
                            v_nat[:, :, h * VW + DH:h * VW + VW], 1.0)

                # ct4 k-chains right after stats (no sigma dependency,
                # covers the serial sigma tails); a_bc broadcasts + tails
                # trickle in as sigma[t] complete
                ps4, ms4 = _emit_ct(4)
                _emit_ab(0)
                _emit_ab(1)
                # epi-t needs ab[t]; ab2/ab3 slot in as epi-t frees banks
                _emit_ct_tail(4, ps4, ms4,
                              after_t=lambda t: _emit_ab(t + 2)
                              if t < 2 else None)
                for ct in CT_ORDER[1:]:
                    ps_c, ms_c = _emit_ct(ct)
                    _emit_ct_tail(ct, ps_c, ms_c)
                    if ct == 5:
                        _emit_vnat(0)
                        _emit_vnat(1)

            # ------- phase C: attention + interleaved out-projection ---------
            with (
                tc.tile_pool(name="pC", bufs=1) as pc,
                tc.tile_pool(name="psC", bufs=1, space="PSUM") as psc,
            ):
                pending_norm = []  # deferred normalizes (drain at jt0)
                pending_op = []    # deferred out-proj tiles (one per j-tile)

                def _norm(P, ib, o_ps, off):
                    """Normalize one head: den bcast, recip, mul."""
                    isl = slice(ib * 512, (ib + 1) * 512)
                    dnb = pc.tile([128, 512], BF16, tag="dnb", bufs=2)
                    nc.vector.tensor_copy(dnb[64:65, :], o_ps[64:65, :])
                    rb_ps = psc.tile([64, 512], F32, tag="rb", bufs=2)
                    nc.tensor.matmul(rb_ps[:], ones_b[64:65, 0:64],
                                     dnb[64:65, :], start=True, stop=True,
                                     skip_group_check=True)
                    rdb = pc.tile([64, 512], F32, tag="rdb", bufs=2)
                    nc.vector.reciprocal_approx_fast(rdb[:], rb_ps[:])
                    if off == 0:
                        nc.vector.tensor_tensor(oT[P][0:64, isl],
                                                o_ps[0:64, :], rdb[:], op=MUL)
                    else:
                        osh = pc.tile([64, 512], BF16, tag="osh", bufs=2)
                        nc.vector.tensor_tensor(osh[:], o_ps[0:64, :],
                                                rdb[:], op=MUL)
                        nc.sync.dma_start(oT[P][64:128, isl], osh[:])

                def _outproj(t, use_act=False):
                    """One 128-token row tile of the partial out-projection.
                    use_act routes the psum->bf16 copy to ScalarE (idle at
                    the kernel tail while DVE finishes the last norms)."""
                    rsl = slice(t * 128, (t + 1) * 128)
                    op = psc.tile([128, 1024], F32, tag="s", bufs=2,
                                  name=f"op{t}")
                    for c in range(2):
                        for mt in range(2):
                            nc.tensor.matmul(
                                op[:, mt * 512:(mt + 1) * 512],
                                oT[c][:, rsl], wo_t[:, c, mt * 512:(mt + 1) * 512],
                                start=(c == 0), stop=(c == 1),
                                skip_group_check=True)
                    ost = pc.tile([128, 1024], BF16, tag="ost", bufs=2,
                                  name=f"ost{t}")
                    if use_act:
                        nc.scalar.activation(ost[:], op[:], AF.Copy)
                    else:
                        nc.vector.tensor_copy(ost[:], op[:])
                    nc.sync.dma_start(out_ext[rsl, :], ost[:])

                for ib in (3, 2, 1, 0):
                    i0 = ib * 512
                    n_jt = 4 * (ib + 1)
                    for P in range(2):
                        qT = qkvT[P]
                        kT = qkvT[2 + P]
                        hA, hB = 2 * P, 2 * P + 1
                        o_A = psc.tile([VW, 512], F32, tag="o", bufs=2,
                                       name=f"oA{ib}{P}")
                        o_B = psc.tile([VW, 512], F32, tag="o", bufs=2,
                                       name=f"oB{ib}{P}")
                        pend_pv = None

                        for jt in range(n_jt):
                            j0 = jt * 128
                            so = max(0, j0 - i0)
                            s2 = psc.tile([128, 1024], F32, tag="s", bufs=2,
                                          name=f"s{ib}{P}{jt}")
                            # QK pair: A rows 0:64 @ pos(0,0), B rows 64:128
                            nc.tensor.matmul(
                                s2[:, so:512], kT[0:64, j0:j0 + 128],
                                qT[0:64, i0 + so:i0 + 512],
                                start=True, stop=True, skip_group_check=True)
                            nc.tensor.matmul(
                                s2[:, 512 + so:1024], kT[64:128, j0:j0 + 128],
                                qT[64:128, i0 + so:i0 + 512],
                                start=True, stop=True, skip_group_check=True)
                            e2 = pc.tile([128, 1024], BF16, tag="e", bufs=3,
                                         name=f"e{ib}{P}{jt}")
                            nc.scalar.activation(e2[:, so:1024],
                                                 s2[:, so:1024], AF.Exp)
                            if j0 >= i0:  # diagonal tile: mask both heads
                                ev = e2[:].rearrange(
                                    "p (a b) -> p a b", b=512)[:, :, so:so + 128]
                                mv = mask_t[:].rearrange(
                                    "p (a b) -> p a b", b=128)
                                nc.vector.tensor_tensor(ev, ev, mv, op=MUL)

                            def _pv(so_, jt_, e2_, first, last):
                                nc.tensor.matmul(
                                    o_A[:, so_:512],
                                    v_nat[:, jt_, hA * VW:hA * VW + VW],
                                    e2_[:, so_:512], start=first, stop=last,
                                    skip_group_check=True)
                                nc.tensor.matmul(
                                    o_B[:, so_:512],
                                    v_nat[:, jt_, hB * VW:hB * VW + VW],
                                    e2_[:, 512 + so_:1024], start=first,
                                    stop=last, skip_group_check=True)

                            if pend_pv is not None:
                                pend_pv()
                            pend_pv = (lambda a=so, b=jt, c=e2,
                                       f=(jt == 0), l=(jt == n_jt - 1):
                                       _pv(a, b, c, f, l))
                            if jt == 0:
                                # norms of the previous block: after this
                                # block's first QK/exp (keeps ScalarE fed),
                                # before its first PV reuses the o-ring slots
                                while pending_norm:
                                    pending_norm.pop(0)()
                            # deferred PE filler, spread thinly so the PE
                            # never starves ScalarE of QK psums
                            elif pending_op and (jt % 4 == 2
                                                 or len(pending_op) > 4):
                                pending_op.pop(0)()
                        pend_pv()
                        pending_norm.append(
                            lambda P_=P, ib_=ib, o_=o_A: _norm(P_, ib_, o_, 0))
                        pending_norm.append(
                            lambda P_=P, ib_=ib, o_=o_B: _norm(P_, ib_, o_, 64))
                    # out-projection row tiles for this i-block (deferred)
                    for t in range(4 * ib, 4 * ib + 4):
                        pending_op.append(
                            lambda t_=t, a=False: _outproj(t_, a))
                while pending_norm:
                    pending_norm.pop(0)()
                while pending_op:
                    t_fn = pending_op.pop(0)
                    t_fn.__defaults__ = (t_fn.__defaults__[0], True)
                    t_fn()

    nc.compile()
    return nc


_NC_CACHE = {}


def _get_nc():
    if "nc" not in _NC_CACHE:
        _NC_CACHE["nc"] = _build()
    return _NC_CACHE["nc"]


def _prep_in_maps(x, ln_w, ln_b, w_qkv, w_out):
    import ml_dtypes
    _bf = ml_dtypes.bfloat16
    x = np.asarray(x, dtype=np.float32)
    ln_w = np.asarray(ln_w, dtype=np.float32)
    ln_b = np.asarray(ln_b, dtype=np.float32)
    w_qkv = np.asarray(w_qkv, dtype=np.float32)
    w_out = np.asarray(w_out, dtype=np.float32)

    ones = np.ones((128, 128), dtype=_bf)
    # mask[jp, ii] = 1 iff jp <= ii (keep j <= i), doubled side by side so a
    # single strided DVE op masks both heads' diagonal tiles
    mask1 = np.triu(np.ones((128, 128), dtype=np.float32))
    mask = np.concatenate([mask1, mask1], axis=1).astype(_bf)
    # block mask: row0 selects head-A denominator for out partitions 0:64,
    # row1 selects head-B for 64:128
    bm = np.zeros((2, 128), dtype=np.float32)
    bm[0, 0:64] = 1.0
    bm[1, 64:128] = 1.0
    bm = bm.astype(_bf)

    xTs = [np.ascontiguousarray(x[b].T).astype(_bf) for b in range(B)]

    in_maps = []
    for core in range(8):
        b, hg = core // 4, core % 4
        csl = slice(hg * CD, (hg + 1) * CD)
        # raw slices with SCALE folded into q
        w0 = np.concatenate([w_qkv[:, csl] * SCALE,
                             w_qkv[:, DIM + hg * CD:DIM + (hg + 1) * CD],
                             w_qkv[:, 2 * DIM + hg * CD:2 * DIM + (hg + 1) * CD]],
                            axis=1)
        wf = ln_w[:, None] * w0                      # ln_w folded
        u = wf.sum(axis=0)                           # pairs with -mean
        vb = ln_b @ w0                               # pairs with std (ln bias)
        uv = np.stack([u, vb]).astype(_bf)
        in_maps.append({
            "xT": xTs[b],
            "wqkv": wf.astype(_bf),
            "uv": uv,
            "wout": np.ascontiguousarray(w_out[csl, :]).astype(_bf),
            "ones": ones,
            "mask": mask,
            "bm": bm,
        })
    return in_maps


def _combine(results):
    out = np.empty((B, N, DIM), dtype=np.float32)
    for b in range(B):
        acc = results[b * 4]["out"].astype(np.float32)
        for hg in range(1, 4):
            acc = acc + results[b * 4 + hg]["out"].astype(np.float32)
        out[b] = acc
    return out


def kernel(x, ln_w, ln_b, w_qkv, w_out):
    nc = _get_nc()
    in_maps = _prep_in_maps(x, ln_w, ln_b, w_qkv, w_out)
    res = run_bass_kernel_spmd(nc, in_maps, core_ids=list(range(8)))
    return _combine(res.results)


def run_traced(x, ln_w, ln_b, w_qkv, w_out, **kwargs):
    """Run with NTFF profiling; returns (output, BassKernelResults)."""
    nc = _get_nc()
    in_maps = _prep_in_maps(x, ln_w, ln_b, w_qkv, w_out)
    res = run_bass_kernel_spmd(nc, in_maps, core_ids=list(range(8)),
                               trace=True, **kwargs)
    return _combine(res.results), res
